# revision 8
# baseline (speedup 1.0000x reference)
"""Dice loss kernel for Trainium2 (8 NeuronCores, SPMD data-parallel).

Problem: nn_DiceLoss — logits [8,19,512,512] f32, targets [8,512,512] int64.
  probs = softmax(logits, axis=1)
  PS[c] = sum_px probs[c,px]            (probs_sum)
  I[c]  = sum_px probs[t(px),px]*[t==c] (intersection)
  CT[c] = histogram(targets)            (counts; computed on host)
  dice  = (2I+1)/(PS+CT+1); loss = mean(1-dice)

Sharding: batch b -> core b.

Key trick: the host SORTS pixels by target class (stable argsort) and pads
each class segment to a multiple of 128 pixels. On device, pixels are laid
out so that each 128-pixel group is one COLUMN of a [128, 512] tile
(partition = px%128, free col = px//128). The PE colsum pass (ones-column
lhsT matmul) then yields per-column sums W_colsum[c, col] that preserve the
class segmentation: I[c] is just the sum of W_colsum[c, cols-of-class-c]
and PS[c] the sum over all columns. No masks, no second elementwise pass.

Padding dummies get logits -10 everywhere except +10 on one known class, so
each dummy contributes exactly 1.0 to that class's PS (host-subtracted) and
~e-20 ~= 0 elsewhere.

Per window h (4 full windows of 65536 px + one 2560-px tail):
  - DMA: 19 contiguous 128KB loads (class c rows)
  - ACT: E = exp(L) bf16, batched 4 classes per instruction
  - PE : S = sum_c E via identity-matmul PSUM accumulation
  - DVE: r = approx-recip(S) f32 -> bf16 (partition-aligned, no broadcast)
  - DVE: W_c = E_c * r (tensor_tensor bf16, 2x mode)
  - PE : colsum matmuls (onescol_c lhsT) into per-window [19, 512] PSUM
  - DMA out: [19, 512] f32 per window -> out [19, 2068]
Host: per-class column-range sums + dummy corrections + dice.
"""

import sys

import numpy as np

sys.path.insert(0, "/opt/trn_rl_repo")

import ml_dtypes  # noqa: E402

B, C, H, W = 8, 19, 512, 512
HW = H * W  # 262144
IGNORE_INDEX = 255
SMOOTH = 1.0

F = 512  # free-dim columns per main tile
PXW = 128 * F  # 65536 pixels per main window
NW = 4  # full windows
TAILF = 20  # tail columns per class (2560 px)
NP = NW * PXW + TAILF * 128  # 264704 padded pixels
UCOLS = NW * F + TAILF  # 2068 total 128-px column units
ROWS_MAIN = C * NW * 128  # 9728
TAIL_COLS = C * TAILF  # 380
CONST_COLS = 128 + C * C

_CACHE = {}


def _host_consts():
    """identity [128,128] + per-class ones-column lhsT variants [128,19]."""
    bf16 = ml_dtypes.bfloat16
    cb = np.zeros((128, CONST_COLS), dtype=bf16)
    cb[:, 0:128] = np.eye(128, dtype=bf16)
    for c in range(C):
        cb[:, 128 + C * c + c] = 1
    return (cb,)


def _class_layout(t_flat):
    """Segment layout for one core: counts, per-class pads, column starts.

    Returns (counts, pad, ucol_start, ucol_len, ps_corr) where ucol_* are in
    128-px column units and ps_corr[c] = exact dummy mass to subtract from
    PS[c].
    """
    valid = t_flat != IGNORE_INDEX
    counts = np.bincount(t_flat[valid].astype(np.int64), minlength=C)[:C]
    pad = (-counts) % 128
    seg = counts + pad
    starts_px = np.concatenate([[0], np.cumsum(seg)])
    assert starts_px[-1] <= NP
    ucol_start = starts_px[:-1] // 128
    ucol_len = seg // 128
    ps_corr = np.zeros(C, dtype=np.float64)
    for c in range(C):
        ps_corr[(c + 1) % C] += pad[c]  # segment-c dummies dump on class c+1
    ps_corr[0] += NP - starts_px[-1]  # trailing dummies dump on class 0
    return counts, pad, ucol_start, ucol_len, ps_corr


def _prep_core(logits_b, t_flat):
    """Sorted+padded device arrays for one core: (main [9728,512], tail [128,380])."""
    bf16 = ml_dtypes.bfloat16
    counts, pad, _, _, _ = _class_layout(t_flat)
    order = np.argsort(t_flat, kind="stable")  # class-sorted; ignored last
    Ls = logits_b.reshape(C, HW)[:, order]

    Lp = np.full((C, NP), -10.0, dtype=np.float32)
    Lp[0, :] = 10.0  # trail default: all dummy mass on class 0
    pos_out = 0
    pos_in = 0
    for c in range(C):
        n = int(counts[c])
        Lp[:, pos_out : pos_out + n] = Ls[:, pos_in : pos_in + n]
        p = int(pad[c])
        if p:
            reg = slice(pos_out + n, pos_out + n + p)
            Lp[:, reg] = -10.0
            Lp[(c + 1) % C, reg] = 10.0
        pos_out += n + p
        pos_in += n

    main = (
        Lp[:, : NW * PXW]
        .reshape(C, NW, F, 128)
        .transpose(0, 1, 3, 2)
        .reshape(ROWS_MAIN, F)
        .astype(bf16)
    )
    tail = (
        Lp[:, NW * PXW :]
        .reshape(C, TAILF, 128)
        .transpose(2, 0, 1)
        .reshape(128, TAIL_COLS)
        .astype(bf16)
    )
    return main, tail


def _build_program():
    import concourse.bacc as bacc
    import concourse.mybir as mybir
    import concourse.tile as tile

    dt = mybir.dt
    AOP = mybir.AluOpType
    ACTF = mybir.ActivationFunctionType

    nc = bacc.Bacc("TRN2", target_bir_lowering=False, debug=False)
    main_d = nc.declare_dram_parameter(
        "logits_main", [ROWS_MAIN, F], dt.bfloat16, isOutput=False
    )
    tail_d = nc.declare_dram_parameter(
        "logits_tail", [128, TAIL_COLS], dt.bfloat16, isOutput=False
    )
    cb_d = nc.declare_dram_parameter(
        "consts_bf", [128, CONST_COLS], dt.bfloat16, isOutput=False
    )
    out_d = nc.declare_dram_parameter("out", [C, UCOLS], dt.float32, isOutput=True)

    NH = NW + 1  # 4 main windows + tail

    with tile.TileContext(nc) as tc:
        with (
            tc.tile_pool(name="singles", bufs=1) as sing,
            tc.tile_pool(name="Lw", bufs=3) as Lwp,
            tc.tile_pool(name="Ew", bufs=3) as Ewp,
            tc.tile_pool(name="Rp", bufs=2) as Rp,
            tc.tile_pool(name="Wp", bufs=4) as Wp,
            tc.tile_pool(name="psS", bufs=3, space="PSUM") as psS,
            tc.tile_pool(name="psW", bufs=2, space="PSUM") as psWp,
        ):
            # preload the Exp activation table off the critical path
            dumm = sing.tile([1, 2], dt.bfloat16)
            nc.vector.memset(dumm[:], 0.0)
            nc.scalar.activation(dumm[:], dumm[:], ACTF.Exp)

            consts = sing.tile([128, CONST_COLS], dt.bfloat16)
            nc.sync.dma_start(consts[:], cb_d[:])
            ident = consts[0:128, 0:128]
            onescol = [consts[0:128, 128 + C * c : 128 + C * (c + 1)] for c in range(C)]
            owb = sing.tile([C, UCOLS], dt.float32)

            Ls = [None] * NH
            Es = [None] * NH
            SPs = [None] * NH
            Rbs = [None] * NH

            def setup(h):
                """DMA loads + exp + S accumulation for window h."""
                if h < NW:
                    L = Lwp.tile([128, C * F], dt.bfloat16, tag="L")
                    E = Ewp.tile([128, C * F], dt.bfloat16, tag="E")
                    SP = psS.tile([128, F], dt.float32, tag="S")
                    for c in range(C):
                        r0 = (c * NW + h) * 128
                        nc.sync.dma_start(
                            L[:, c * F : (c + 1) * F], main_d[r0 : r0 + 128, :]
                        )
                    for c0 in range(0, C, 4):
                        c1 = min(c0 + 4, C)
                        nc.scalar.activation(
                            E[:, c0 * F : c1 * F], L[:, c0 * F : c1 * F], ACTF.Exp
                        )
                    for c in range(C):
                        nc.tensor.matmul(
                            SP[:],
                            ident,
                            E[:, c * F : (c + 1) * F],
                            start=(c == 0),
                            stop=(c == C - 1),
                        )
                else:
                    L = Lwp.tile([128, TAIL_COLS], dt.bfloat16, tag="Lt", bufs=1)
                    E = Ewp.tile([128, TAIL_COLS], dt.bfloat16, tag="Et", bufs=1)
                    SP = psS.tile([128, TAILF], dt.float32, tag="St", bufs=1)
                    nc.sync.dma_start(L[:], tail_d[:])
                    nc.scalar.activation(E[:], L[:], ACTF.Exp)
                    for c in range(C):
                        nc.tensor.matmul(
                            SP[:],
                            ident,
                            E[:, c * TAILF : (c + 1) * TAILF],
                            start=(c == 0),
                            stop=(c == C - 1),
                        )
                Ls[h], Es[h], SPs[h] = L, E, SP

            def recip(h):
                f = F if h < NW else TAILF
                nb = 2 if h < NW else 1
                Rf = Rp.tile([128, f], dt.float32, tag=f"Rf{h < NW}", bufs=nb)
                nc.vector.reciprocal_approx_fast(Rf[:], SPs[h][:])
                Rb = Rp.tile([128, f], dt.bfloat16, tag=f"Rb{h < NW}", bufs=nb)
                nc.vector.tensor_copy(Rb[:], Rf[:])
                Rbs[h] = Rb

            def colsum(h):
                f = F if h < NW else TAILF
                nb = 4 if h < NW else 2
                E, Rb = Es[h], Rbs[h]
                psW = psWp.tile(
                    [C, f], dt.float32, tag=f"psW{h < NW}", bufs=min(nb, 2)
                )
                for c in range(C):
                    Wt = Wp.tile([128, f], dt.bfloat16, tag=f"W{h < NW}", bufs=nb)
                    nc.vector.tensor_tensor(
                        out=Wt[:], in0=E[:, c * f : (c + 1) * f], in1=Rb[:], op=AOP.mult
                    )
                    nc.tensor.matmul(
                        psW[:], onescol[c], Wt[:], start=(c == 0), stop=(c == C - 1)
                    )
                u0 = h * F if h < NW else NW * F
                nc.vector.tensor_copy(owb[:, u0 : u0 + f], psW[:])
                nc.sync.dma_start(out_d[0:C, u0 : u0 + f], owb[:, u0 : u0 + f])

            # software pipeline, tail window first (cheap PE warm-up while
            # window-0 DMAs land); stay 2-3 setups ahead of the colsum pass
            # so PE never waits on the DVE recip/W chain
            ws = [NW, 0, 1, 2, 3]
            setup(ws[0])
            setup(ws[1])
            setup(ws[2])
            recip(ws[0])
            for i, h in enumerate(ws):
                colsum(h)
                if i + 3 < NH:
                    setup(ws[i + 3])
                if i + 1 < NH:
                    recip(ws[i + 1])

    nc.compile()
    return nc


def _get_program():
    if "nc" not in _CACHE:
        _CACHE["nc"] = _build_program()
        _CACHE["consts"] = _host_consts()
    return _CACHE["nc"], _CACHE["consts"]


def _install_ntff_hook():
    """antenv.axon_hooks is missing in this image; synthesize it so
    run_bass_kernel_spmd(trace=True) can capture NTFF profiles via axon."""
    import types

    if "antenv.axon_hooks" in sys.modules:
        return
    mod = types.ModuleType("antenv.axon_hooks")
    _h = [None]
    mod.set_axon_ntff_profile_hook = lambda h: _h.__setitem__(0, h)
    mod.get_axon_ntff_profile_hook = lambda: _h[0]
    sys.modules["antenv.axon_hooks"] = mod
    import antenv

    antenv.axon_hooks = mod
    from trn_agent_boot.trn_boot import _ntff_profile_via_ctypes

    mod.set_axon_ntff_profile_hook(
        _ntff_profile_via_ctypes("/opt/axon/libaxon_pjrt.so")
    )


def _run_device(logits_np, targets_np, trace=False):
    """Run the SPMD kernel on 8 cores; returns (list of out arrays, results obj)."""
    from concourse.bass_utils import run_bass_kernel_spmd

    nc, (cb,) = _get_program()
    logits_np = np.asarray(logits_np, dtype=np.float32)
    targets_np = np.asarray(targets_np)
    in_maps = []
    for b in range(B):
        main, tail = _prep_core(logits_np[b], targets_np[b].reshape(-1))
        in_maps.append({"logits_main": main, "logits_tail": tail, "consts_bf": cb})
    kwargs = {}
    if trace:
        _install_ntff_hook()
        kwargs = {"trace": True, "trace_cores": [0]}
    res = run_bass_kernel_spmd(nc, in_maps, core_ids=list(range(B)), **kwargs)
    outs = [res.results[b]["out"] for b in range(B)]
    return outs, res


def _combine(outs, targets_np):
    targets_np = np.asarray(targets_np)
    t_all = targets_np.reshape(-1)
    valid_all = t_all != IGNORE_INDEX
    if not valid_all.any():
        return np.asarray(0.0, dtype=np.float32)
    PS = np.zeros(C, dtype=np.float64)
    I = np.zeros(C, dtype=np.float64)
    for b, o in enumerate(outs):
        psw = o.astype(np.float64)  # [C, UCOLS] per-column sums of W_c
        t_flat = targets_np[b].reshape(-1)
        _, _, ustart, ulen, ps_corr = _class_layout(t_flat)
        PS += psw.sum(axis=1) - ps_corr
        for c in range(C):
            I[c] += psw[c, ustart[c] : ustart[c] + ulen[c]].sum()
    CT = np.bincount(t_all[valid_all].astype(np.int64), minlength=C)[:C].astype(
        np.float64
    )
    dice = (2.0 * I + SMOOTH) / (PS + CT + SMOOTH)
    loss = (1.0 - dice).mean()
    return np.asarray(loss, dtype=np.float32)


def kernel(logits, targets):
    logits = np.asarray(logits)
    targets = np.asarray(targets)
    outs, _ = _run_device(logits, targets)
    return _combine(outs, targets)


# revision 14
# speedup vs baseline: 1.0151x; 1.0151x over previous
"""Dice loss kernel for Trainium2 (8 NeuronCores, SPMD data-parallel).

Problem: nn_DiceLoss — logits [8,19,512,512] f32, targets [8,512,512] int64.
  probs = softmax(logits, axis=1)
  PS[c] = sum_px probs[c,px]            (probs_sum)
  I[c]  = sum_px probs[t(px),px]*[t==c] (intersection)
  CT[c] = histogram(targets)            (counts; computed on host)
  dice  = (2I+1)/(PS+CT+1); loss = mean(1-dice)

Sharding: batch b -> core b.

Key trick: the host SORTS pixels by target class (stable argsort) and pads
each class segment to a multiple of 128 pixels. On device, pixels are laid
out so that each 128-pixel group is one COLUMN of a [128, 512] tile
(partition = px%128, free col = px//128). The PE colsum pass (ones-column
lhsT matmul) then yields per-column sums W_colsum[c, col] that preserve the
class segmentation: I[c] is just the sum of W_colsum[c, cols-of-class-c]
and PS[c] the sum over all columns. No masks, no second elementwise pass.

Padding dummies get logits -10 everywhere except +10 on one known class, so
each dummy contributes exactly 1.0 to that class's PS (host-subtracted) and
~e-20 ~= 0 elsewhere.

Per window h (4 full windows of 65536 px + one 2560-px tail):
  - DMA: 19 contiguous 128KB loads (class c rows)
  - ACT: E = exp(L) bf16, batched 4 classes per instruction
  - PE : S = sum_c E via identity-matmul PSUM accumulation
  - DVE: r = approx-recip(S) f32 -> bf16 (partition-aligned, no broadcast)
  - DVE: W_c = E_c * r (tensor_tensor bf16, 2x mode)
  - PE : colsum matmuls (onescol_c lhsT) into per-window [19, 512] PSUM
  - DMA out: [19, 512] f32 per window -> out [19, 2068]
Host: per-class column-range sums + dummy corrections + dice.
"""

import sys

import numpy as np

sys.path.insert(0, "/opt/trn_rl_repo")

import ml_dtypes  # noqa: E402

B, C, H, W = 8, 19, 512, 512
HW = H * W  # 262144
IGNORE_INDEX = 255
SMOOTH = 1.0

F = 512  # free-dim columns per main tile
PXW = 128 * F  # 65536 pixels per main window
NW = 4  # full windows
TAILF = 20  # tail columns per class (2560 px)
NP = NW * PXW + TAILF * 128  # 264704 padded pixels
UCOLS = NW * F + TAILF  # 2068 total 128-px column units
ROWS_MAIN = C * NW * 128  # 9728
TAIL_COLS = C * TAILF  # 380
CONST_COLS = 128 + C * C

_CACHE = {}


def _host_consts():
    """identity [128,128] + per-class ones-column lhsT variants [128,19]."""
    bf16 = ml_dtypes.bfloat16
    cb = np.zeros((128, CONST_COLS), dtype=bf16)
    cb[:, 0:128] = np.eye(128, dtype=bf16)
    for c in range(C):
        cb[:, 128 + C * c + c] = 1
    return (cb,)


def _class_layout(t_flat):
    """Segment layout for one core: counts, per-class pads, column starts.

    Returns (counts, pad, ucol_start, ucol_len, ps_corr) where ucol_* are in
    128-px column units and ps_corr[c] = exact dummy mass to subtract from
    PS[c].
    """
    valid = t_flat != IGNORE_INDEX
    counts = np.bincount(t_flat[valid].astype(np.int64), minlength=C)[:C]
    pad = (-counts) % 128
    seg = counts + pad
    starts_px = np.concatenate([[0], np.cumsum(seg)])
    assert starts_px[-1] <= NP
    ucol_start = starts_px[:-1] // 128
    ucol_len = seg // 128
    ps_corr = np.zeros(C, dtype=np.float64)
    for c in range(C):
        ps_corr[(c + 1) % C] += pad[c]  # segment-c dummies dump on class c+1
    ps_corr[0] += NP - starts_px[-1]  # trailing dummies dump on class 0
    return counts, pad, ucol_start, ucol_len, ps_corr


def _prep_core(logits_b, t_flat):
    """Sorted+padded device arrays for one core: (main [9728,512], tail [128,380])."""
    bf16 = ml_dtypes.bfloat16
    counts, pad, _, _, _ = _class_layout(t_flat)
    order = np.argsort(t_flat, kind="stable")  # class-sorted; ignored last
    Ls = logits_b.reshape(C, HW)[:, order]

    Lp = np.full((C, NP), -10.0, dtype=np.float32)
    Lp[0, :] = 10.0  # trail default: all dummy mass on class 0
    pos_out = 0
    pos_in = 0
    for c in range(C):
        n = int(counts[c])
        Lp[:, pos_out : pos_out + n] = Ls[:, pos_in : pos_in + n]
        p = int(pad[c])
        if p:
            reg = slice(pos_out + n, pos_out + n + p)
            Lp[:, reg] = -10.0
            Lp[(c + 1) % C, reg] = 10.0
        pos_out += n + p
        pos_in += n

    main = (
        Lp[:, : NW * PXW]
        .reshape(C, NW, F, 128)
        .transpose(0, 1, 3, 2)
        .reshape(ROWS_MAIN, F)
        .astype(bf16)
    )
    tail = (
        Lp[:, NW * PXW :]
        .reshape(C, TAILF, 128)
        .transpose(2, 0, 1)
        .reshape(128, TAIL_COLS)
        .astype(bf16)
    )
    return main, tail


def _build_program():
    import concourse.bacc as bacc
    import concourse.mybir as mybir
    import concourse.tile as tile

    dt = mybir.dt
    AOP = mybir.AluOpType
    ACTF = mybir.ActivationFunctionType

    nc = bacc.Bacc("TRN2", target_bir_lowering=False, debug=False)
    main_d = nc.declare_dram_parameter(
        "logits_main", [ROWS_MAIN, F], dt.bfloat16, isOutput=False
    )
    tail_d = nc.declare_dram_parameter(
        "logits_tail", [128, TAIL_COLS], dt.bfloat16, isOutput=False
    )
    cb_d = nc.declare_dram_parameter(
        "consts_bf", [128, CONST_COLS], dt.bfloat16, isOutput=False
    )
    out_d = nc.declare_dram_parameter("out", [C, UCOLS], dt.float32, isOutput=True)

    NH = NW + 1  # 4 main windows + tail

    with tile.TileContext(nc) as tc:
        with (
            tc.tile_pool(name="singles", bufs=1) as sing,
            tc.tile_pool(name="Ew", bufs=4) as Ewp,
            tc.tile_pool(name="Rp", bufs=2) as Rp,
            tc.tile_pool(name="Wp", bufs=2) as Wp,
            tc.tile_pool(name="psS", bufs=3, space="PSUM") as psS,
            tc.tile_pool(name="psW", bufs=2, space="PSUM") as psWp,
        ):
            # preload the Exp activation table off the critical path
            dumm = sing.tile([1, 2], dt.bfloat16)
            nc.vector.memset(dumm[:], 0.0)
            nc.scalar.activation(dumm[:], dumm[:], ACTF.Exp)

            consts = sing.tile([128, CONST_COLS], dt.bfloat16)
            nc.sync.dma_start(consts[:], cb_d[:])
            ident = consts[0:128, 0:128]
            onescol = [consts[0:128, 128 + C * c : 128 + C * (c + 1)] for c in range(C)]
            owb = sing.tile([C, UCOLS], dt.float32)

            Es = [None] * NH
            SPs = [None] * NH
            Rbs = [None] * NH

            def setup(h):
                """DMA loads + in-place exp + S accumulation for window h."""
                if h < NW:
                    E = Ewp.tile([128, C * F], dt.bfloat16, tag="E")
                    SP = psS.tile([128, F], dt.float32, tag="S")
                    for c in range(C):
                        r0 = (c * NW + h) * 128
                        nc.sync.dma_start(
                            E[:, c * F : (c + 1) * F], main_d[r0 : r0 + 128, :]
                        )
                    for c0 in range(0, C, 4):
                        c1 = min(c0 + 4, C)
                        nc.scalar.activation(
                            E[:, c0 * F : c1 * F], E[:, c0 * F : c1 * F], ACTF.Exp
                        )
                    for c in range(C):
                        nc.tensor.matmul(
                            SP[:],
                            ident,
                            E[:, c * F : (c + 1) * F],
                            start=(c == 0),
                            stop=(c == C - 1),
                        )
                else:
                    E = Ewp.tile([128, TAIL_COLS], dt.bfloat16, tag="Et", bufs=1)
                    SP = psS.tile([128, TAILF], dt.float32, tag="St", bufs=1)
                    nc.sync.dma_start(E[:], tail_d[:])
                    nc.scalar.activation(E[:], E[:], ACTF.Exp)
                    for c in range(C):
                        nc.tensor.matmul(
                            SP[:],
                            ident,
                            E[:, c * TAILF : (c + 1) * TAILF],
                            start=(c == 0),
                            stop=(c == C - 1),
                        )
                Es[h], SPs[h] = E, SP

            def recip(h):
                f = F if h < NW else TAILF
                nb = 2 if h < NW else 1
                Rf = Rp.tile([128, f], dt.float32, tag=f"Rf{h < NW}", bufs=nb)
                nc.vector.reciprocal_approx_fast(Rf[:], SPs[h][:])
                Rb = Rp.tile([128, f], dt.bfloat16, tag=f"Rb{h < NW}", bufs=nb)
                nc.vector.tensor_copy(Rb[:], Rf[:])
                Rbs[h] = Rb

            GRP = 4  # classes per grouped W multiply

            def colsum(h):
                f = F if h < NW else TAILF
                E, Rb = Es[h], Rbs[h]
                psW = psWp.tile([C, f], dt.float32, tag=f"psW{h < NW}", bufs=2)
                for c0 in range(0, C, GRP):
                    c1 = min(c0 + GRP, C)
                    g = c1 - c0
                    Wt = Wp.tile([128, GRP * f], dt.bfloat16, tag=f"W{h < NW}")
                    nc.vector.tensor_tensor(
                        out=Wt[:, 0 : g * f].rearrange("p (g f) -> p g f", f=f),
                        in0=E[:, c0 * f : c1 * f].rearrange("p (g f) -> p g f", f=f),
                        in1=Rb[:].unsqueeze(1).broadcast_to([128, g, f]),
                        op=AOP.mult,
                    )
                    for c in range(c0, c1):
                        nc.tensor.matmul(
                            psW[:],
                            onescol[c],
                            Wt[:, (c - c0) * f : (c - c0 + 1) * f],
                            start=(c == 0),
                            stop=(c == C - 1),
                        )
                u0 = h * F if h < NW else NW * F
                nc.vector.tensor_copy(owb[:, u0 : u0 + f], psW[:])
                nc.sync.dma_start(out_d[0:C, u0 : u0 + f], owb[:, u0 : u0 + f])

            # software pipeline: stay 2-3 setups ahead of the colsum pass so
            # PE never waits on the DVE recip/W chain; tiny tail drains last
            ws = [0, 1, 2, 3, NW]
            setup(ws[0])
            setup(ws[1])
            setup(ws[2])
            recip(ws[0])
            for i, h in enumerate(ws):
                colsum(h)
                if i + 3 < NH:
                    setup(ws[i + 3])
                if i + 1 < NH:
                    recip(ws[i + 1])

    nc.compile()
    return nc


def _get_program():
    if "nc" not in _CACHE:
        _CACHE["nc"] = _build_program()
        _CACHE["consts"] = _host_consts()
    return _CACHE["nc"], _CACHE["consts"]


def _install_ntff_hook():
    """antenv.axon_hooks is missing in this image; synthesize it so
    run_bass_kernel_spmd(trace=True) can capture NTFF profiles via axon."""
    import types

    if "antenv.axon_hooks" in sys.modules:
        return
    mod = types.ModuleType("antenv.axon_hooks")
    _h = [None]
    mod.set_axon_ntff_profile_hook = lambda h: _h.__setitem__(0, h)
    mod.get_axon_ntff_profile_hook = lambda: _h[0]
    sys.modules["antenv.axon_hooks"] = mod
    import antenv

    antenv.axon_hooks = mod
    from trn_agent_boot.trn_boot import _ntff_profile_via_ctypes

    mod.set_axon_ntff_profile_hook(
        _ntff_profile_via_ctypes("/opt/axon/libaxon_pjrt.so")
    )


def _run_device(logits_np, targets_np, trace=False):
    """Run the SPMD kernel on 8 cores; returns (list of out arrays, results obj)."""
    from concourse.bass_utils import run_bass_kernel_spmd

    nc, (cb,) = _get_program()
    logits_np = np.asarray(logits_np, dtype=np.float32)
    targets_np = np.asarray(targets_np)
    in_maps = []
    for b in range(B):
        main, tail = _prep_core(logits_np[b], targets_np[b].reshape(-1))
        in_maps.append({"logits_main": main, "logits_tail": tail, "consts_bf": cb})
    kwargs = {}
    if trace:
        _install_ntff_hook()
        kwargs = {"trace": True, "trace_cores": [0]}
    res = run_bass_kernel_spmd(nc, in_maps, core_ids=list(range(B)), **kwargs)
    outs = [res.results[b]["out"] for b in range(B)]
    return outs, res


def _combine(outs, targets_np):
    targets_np = np.asarray(targets_np)
    t_all = targets_np.reshape(-1)
    valid_all = t_all != IGNORE_INDEX
    if not valid_all.any():
        return np.asarray(0.0, dtype=np.float32)
    PS = np.zeros(C, dtype=np.float64)
    I = np.zeros(C, dtype=np.float64)
    for b, o in enumerate(outs):
        psw = o.astype(np.float64)  # [C, UCOLS] per-column sums of W_c
        t_flat = targets_np[b].reshape(-1)
        _, _, ustart, ulen, ps_corr = _class_layout(t_flat)
        PS += psw.sum(axis=1) - ps_corr
        for c in range(C):
            I[c] += psw[c, ustart[c] : ustart[c] + ulen[c]].sum()
    CT = np.bincount(t_all[valid_all].astype(np.int64), minlength=C)[:C].astype(
        np.float64
    )
    dice = (2.0 * I + SMOOTH) / (PS + CT + SMOOTH)
    loss = (1.0 - dice).mean()
    return np.asarray(loss, dtype=np.float32)


def kernel(logits, targets):
    logits = np.asarray(logits)
    targets = np.asarray(targets)
    outs, _ = _run_device(logits, targets)
    return _combine(outs, targets)


# revision 17
# speedup vs baseline: 1.1201x; 1.1035x over previous
"""Dice loss kernel for Trainium2 (8 NeuronCores, SPMD data-parallel).

Problem: nn_DiceLoss — logits [8,19,512,512] f32, targets [8,512,512] int64.
  probs = softmax(logits, axis=1)
  PS[c] = sum_px probs[c,px]            (probs_sum)
  I[c]  = sum_px probs[t(px),px]*[t==c] (intersection)
  CT[c] = histogram(targets)            (counts; computed on host)
  dice  = (2I+1)/(PS+CT+1); loss = mean(1-dice)

Sharding: batch b -> core b.

Key trick: the host SORTS pixels by target class (stable argsort) and pads
each class segment to a multiple of 128 pixels. On device, pixels are laid
out so that each 128-pixel group is one COLUMN of a [128, 512] tile
(partition = px%128, free col = px//128). The PE colsum pass (ones-column
lhsT matmul) then yields per-column sums W_colsum[c, col] that preserve the
class segmentation: I[c] is just the sum of W_colsum[c, cols-of-class-c]
and PS[c] the sum over all columns. No masks, no second elementwise pass.

Padding dummies get logits -10 everywhere except +10 on one known class, so
each dummy contributes exactly 1.0 to that class's PS (host-subtracted) and
~e-20 ~= 0 elsewhere.

Per window h (4 full windows of 65536 px + one 2560-px tail):
  - DMA: 19 contiguous 128KB loads (class c rows)
  - ACT: E = exp(L) bf16, batched 4 classes per instruction
  - PE : S = sum_c E via identity-matmul PSUM accumulation
  - DVE: r = approx-recip(S) f32 -> bf16 (partition-aligned, no broadcast)
  - DVE: W_c = E_c * r (tensor_tensor bf16, 2x mode)
  - PE : colsum matmuls (onescol_c lhsT) into per-window [19, 512] PSUM
  - DMA out: [19, 512] f32 per window -> out [19, 2068]
Host: per-class column-range sums + dummy corrections + dice.
"""

import sys

import numpy as np

sys.path.insert(0, "/opt/trn_rl_repo")

import ml_dtypes  # noqa: E402

B, C, H, W = 8, 19, 512, 512
HW = H * W  # 262144
IGNORE_INDEX = 255
SMOOTH = 1.0

F = 512  # free-dim columns per main tile
PXW = 128 * F  # 65536 pixels per main window
NW = 4  # full windows
TAILF = 20  # tail columns per class (2560 px)
NP = NW * PXW + TAILF * 128  # 264704 padded pixels
UCOLS = NW * F + TAILF  # 2068 total 128-px column units
ROWS_MAIN = C * NW * 128  # 9728
TAIL_COLS = C * TAILF  # 380
CONST_COLS = 128 + C * C

_CACHE = {}


def _host_consts():
    """identity [128,128] + per-class ones-column lhsT variants [128,19]."""
    bf16 = ml_dtypes.bfloat16
    cb = np.zeros((128, CONST_COLS), dtype=bf16)
    cb[:, 0:128] = np.eye(128, dtype=bf16)
    for c in range(C):
        cb[:, 128 + C * c + c] = 1
    return (cb,)


def _class_layout(t_flat):
    """Segment layout for one core: counts, per-class pads, column starts.

    Returns (counts, pad, ucol_start, ucol_len, ps_corr) where ucol_* are in
    128-px column units and ps_corr[c] = exact dummy mass to subtract from
    PS[c].
    """
    valid = t_flat != IGNORE_INDEX
    counts = np.bincount(t_flat[valid].astype(np.int64), minlength=C)[:C]
    pad = (-counts) % 128
    seg = counts + pad
    starts_px = np.concatenate([[0], np.cumsum(seg)])
    assert starts_px[-1] <= NP
    ucol_start = starts_px[:-1] // 128
    ucol_len = seg // 128
    ps_corr = np.zeros(C, dtype=np.float64)
    for c in range(C):
        ps_corr[(c + 1) % C] += pad[c]  # segment-c dummies dump on class c+1
    ps_corr[0] += NP - starts_px[-1]  # trailing dummies dump on class 0
    return counts, pad, ucol_start, ucol_len, ps_corr


def _prep_core(logits_b, t_flat):
    """Sorted+padded device arrays for one core: (main [9728,512], tail [128,380])."""
    bf16 = ml_dtypes.bfloat16
    counts, pad, _, _, _ = _class_layout(t_flat)
    order = np.argsort(t_flat, kind="stable")  # class-sorted; ignored last
    Ls = logits_b.reshape(C, HW)[:, order]

    Lp = np.full((C, NP), -10.0, dtype=np.float32)
    Lp[0, :] = 10.0  # trail default: all dummy mass on class 0
    pos_out = 0
    pos_in = 0
    for c in range(C):
        n = int(counts[c])
        Lp[:, pos_out : pos_out + n] = Ls[:, pos_in : pos_in + n]
        p = int(pad[c])
        if p:
            reg = slice(pos_out + n, pos_out + n + p)
            Lp[:, reg] = -10.0
            Lp[(c + 1) % C, reg] = 10.0
        pos_out += n + p
        pos_in += n

    # row (h, p), cols (c, f): one window = one contiguous [128, C*F] DMA
    main = (
        Lp[:, : NW * PXW]
        .reshape(C, NW, F, 128)
        .transpose(1, 3, 0, 2)
        .reshape(NW * 128, C * F)
        .astype(bf16)
    )
    tail = (
        Lp[:, NW * PXW :]
        .reshape(C, TAILF, 128)
        .transpose(2, 0, 1)
        .reshape(128, TAIL_COLS)
        .astype(bf16)
    )
    return main, tail


def _build_program():
    import concourse.bacc as bacc
    import concourse.mybir as mybir
    import concourse.tile as tile

    dt = mybir.dt
    AOP = mybir.AluOpType
    ACTF = mybir.ActivationFunctionType

    nc = bacc.Bacc("TRN2", target_bir_lowering=False, debug=False)
    main_d = nc.declare_dram_parameter(
        "logits_main", [NW * 128, C * F], dt.bfloat16, isOutput=False
    )
    tail_d = nc.declare_dram_parameter(
        "logits_tail", [128, TAIL_COLS], dt.bfloat16, isOutput=False
    )
    cb_d = nc.declare_dram_parameter(
        "consts_bf", [128, CONST_COLS], dt.bfloat16, isOutput=False
    )
    out_d = nc.declare_dram_parameter("out", [C, UCOLS], dt.float32, isOutput=True)

    NH = NW + 1  # 4 main windows + tail

    with tile.TileContext(nc) as tc:
        with (
            tc.tile_pool(name="singles", bufs=1) as sing,
            tc.tile_pool(name="Ew", bufs=4) as Ewp,
            tc.tile_pool(name="Rp", bufs=2) as Rp,
            tc.tile_pool(name="Wp", bufs=2) as Wp,
            tc.tile_pool(name="psS", bufs=3, space="PSUM") as psS,
            tc.tile_pool(name="psW", bufs=2, space="PSUM") as psWp,
        ):
            # preload the Exp activation table off the critical path
            dumm = sing.tile([1, 2], dt.bfloat16)
            nc.vector.memset(dumm[:], 0.0)
            nc.scalar.activation(dumm[:], dumm[:], ACTF.Exp)

            consts = sing.tile([128, CONST_COLS], dt.bfloat16)
            nc.sync.dma_start(consts[:], cb_d[:])
            ident = consts[0:128, 0:128]
            onescol = [consts[0:128, 128 + C * c : 128 + C * (c + 1)] for c in range(C)]
            owb = sing.tile([C, UCOLS], dt.float32)

            Es = [None] * NH
            SPs = [None] * NH
            Rbs = [None] * NH

            def setup(h):
                """DMA loads + in-place exp + S accumulation for window h."""
                if h < NW:
                    E = Ewp.tile([128, C * F], dt.bfloat16, tag="E")
                    SP = psS.tile([128, F], dt.float32, tag="S")
                    nc.sync.dma_start(E[:], main_d[h * 128 : (h + 1) * 128, :])
                    for c0 in range(0, C, 4):
                        c1 = min(c0 + 4, C)
                        nc.scalar.activation(
                            E[:, c0 * F : c1 * F], E[:, c0 * F : c1 * F], ACTF.Exp
                        )
                    for c in range(C):
                        nc.tensor.matmul(
                            SP[:],
                            ident,
                            E[:, c * F : (c + 1) * F],
                            start=(c == 0),
                            stop=(c == C - 1),
                        )
                else:
                    E = Ewp.tile([128, TAIL_COLS], dt.bfloat16, tag="Et", bufs=1)
                    SP = psS.tile([128, TAILF], dt.float32, tag="St", bufs=1)
                    nc.sync.dma_start(E[:], tail_d[:])
                    nc.scalar.activation(E[:], E[:], ACTF.Exp)
                    for c in range(C):
                        nc.tensor.matmul(
                            SP[:],
                            ident,
                            E[:, c * TAILF : (c + 1) * TAILF],
                            start=(c == 0),
                            stop=(c == C - 1),
                        )
                Es[h], SPs[h] = E, SP

            def recip(h):
                f = F if h < NW else TAILF
                nb = 2 if h < NW else 1
                Rf = Rp.tile([128, f], dt.float32, tag=f"Rf{h < NW}", bufs=nb)
                nc.vector.reciprocal_approx_fast(Rf[:], SPs[h][:])
                Rb = Rp.tile([128, f], dt.bfloat16, tag=f"Rb{h < NW}", bufs=nb)
                nc.vector.tensor_copy(Rb[:], Rf[:])
                Rbs[h] = Rb

            GRP = 4  # classes per grouped W multiply

            def colsum(h):
                f = F if h < NW else TAILF
                E, Rb = Es[h], Rbs[h]
                psW = psWp.tile([C, f], dt.float32, tag=f"psW{h < NW}", bufs=2)
                for c0 in range(0, C, GRP):
                    c1 = min(c0 + GRP, C)
                    g = c1 - c0
                    Wt = Wp.tile([128, GRP * f], dt.bfloat16, tag=f"W{h < NW}")
                    nc.vector.tensor_tensor(
                        out=Wt[:, 0 : g * f].rearrange("p (g f) -> p g f", f=f),
                        in0=E[:, c0 * f : c1 * f].rearrange("p (g f) -> p g f", f=f),
                        in1=Rb[:].unsqueeze(1).broadcast_to([128, g, f]),
                        op=AOP.mult,
                    )
                    for c in range(c0, c1):
                        nc.tensor.matmul(
                            psW[:],
                            onescol[c],
                            Wt[:, (c - c0) * f : (c - c0 + 1) * f],
                            start=(c == 0),
                            stop=(c == C - 1),
                        )
                u0 = h * F if h < NW else NW * F
                nc.vector.tensor_copy(owb[:, u0 : u0 + f], psW[:])
                nc.sync.dma_start(out_d[0:C, u0 : u0 + f], owb[:, u0 : u0 + f])

            # software pipeline: stay 2-3 setups ahead of the colsum pass so
            # PE never waits on the DVE recip/W chain; tiny tail drains last
            ws = [0, 1, 2, 3, NW]
            setup(ws[0])
            setup(ws[1])
            setup(ws[2])
            recip(ws[0])
            for i, h in enumerate(ws):
                colsum(h)
                if i + 3 < NH:
                    setup(ws[i + 3])
                if i + 1 < NH:
                    recip(ws[i + 1])

    nc.compile()
    return nc


def _get_program():
    if "nc" not in _CACHE:
        _CACHE["nc"] = _build_program()
        _CACHE["consts"] = _host_consts()
    return _CACHE["nc"], _CACHE["consts"]


def _install_ntff_hook():
    """antenv.axon_hooks is missing in this image; synthesize it so
    run_bass_kernel_spmd(trace=True) can capture NTFF profiles via axon."""
    import types

    if "antenv.axon_hooks" in sys.modules:
        return
    mod = types.ModuleType("antenv.axon_hooks")
    _h = [None]
    mod.set_axon_ntff_profile_hook = lambda h: _h.__setitem__(0, h)
    mod.get_axon_ntff_profile_hook = lambda: _h[0]
    sys.modules["antenv.axon_hooks"] = mod
    import antenv

    antenv.axon_hooks = mod
    from trn_agent_boot.trn_boot import _ntff_profile_via_ctypes

    mod.set_axon_ntff_profile_hook(
        _ntff_profile_via_ctypes("/opt/axon/libaxon_pjrt.so")
    )


def _run_device(logits_np, targets_np, trace=False):
    """Run the SPMD kernel on 8 cores; returns (list of out arrays, results obj)."""
    from concourse.bass_utils import run_bass_kernel_spmd

    nc, (cb,) = _get_program()
    logits_np = np.asarray(logits_np, dtype=np.float32)
    targets_np = np.asarray(targets_np)
    in_maps = []
    for b in range(B):
        main, tail = _prep_core(logits_np[b], targets_np[b].reshape(-1))
        in_maps.append({"logits_main": main, "logits_tail": tail, "consts_bf": cb})
    kwargs = {}
    if trace:
        _install_ntff_hook()
        kwargs = {"trace": True, "trace_cores": [0]}
    res = run_bass_kernel_spmd(nc, in_maps, core_ids=list(range(B)), **kwargs)
    outs = [res.results[b]["out"] for b in range(B)]
    return outs, res


def _combine(outs, targets_np):
    targets_np = np.asarray(targets_np)
    t_all = targets_np.reshape(-1)
    valid_all = t_all != IGNORE_INDEX
    if not valid_all.any():
        return np.asarray(0.0, dtype=np.float32)
    PS = np.zeros(C, dtype=np.float64)
    I = np.zeros(C, dtype=np.float64)
    for b, o in enumerate(outs):
        psw = o.astype(np.float64)  # [C, UCOLS] per-column sums of W_c
        t_flat = targets_np[b].reshape(-1)
        _, _, ustart, ulen, ps_corr = _class_layout(t_flat)
        PS += psw.sum(axis=1) - ps_corr
        for c in range(C):
            I[c] += psw[c, ustart[c] : ustart[c] + ulen[c]].sum()
    CT = np.bincount(t_all[valid_all].astype(np.int64), minlength=C)[:C].astype(
        np.float64
    )
    dice = (2.0 * I + SMOOTH) / (PS + CT + SMOOTH)
    loss = (1.0 - dice).mean()
    return np.asarray(loss, dtype=np.float32)


def kernel(logits, targets):
    logits = np.asarray(logits)
    targets = np.asarray(targets)
    outs, _ = _run_device(logits, targets)
    return _combine(outs, targets)


# revision 19
# speedup vs baseline: 1.2351x; 1.1027x over previous
"""Dice loss kernel for Trainium2 (8 NeuronCores, SPMD data-parallel).

Problem: nn_DiceLoss — logits [8,19,512,512] f32, targets [8,512,512] int64.
  probs = softmax(logits, axis=1)
  PS[c] = sum_px probs[c,px]            (probs_sum)
  I[c]  = sum_px probs[t(px),px]*[t==c] (intersection)
  CT[c] = histogram(targets)            (counts; computed on host)
  dice  = (2I+1)/(PS+CT+1); loss = mean(1-dice)

Sharding: batch b -> core b.

Key trick: the host SORTS pixels by target class (stable argsort) and pads
each class segment to a multiple of 128 pixels. On device, pixels are laid
out so that each 128-pixel group is one COLUMN of a [128, 512] tile
(partition = px%128, free col = px//128). The PE colsum pass (ones-column
lhsT matmul) then yields per-column sums W_colsum[c, col] that preserve the
class segmentation: I[c] is just the sum of W_colsum[c, cols-of-class-c]
and PS[c] the sum over all columns. No masks, no second elementwise pass.

Padding dummies get logits -10 everywhere except +10 on one known class, so
each dummy contributes exactly 1.0 to that class's PS (host-subtracted) and
~e-20 ~= 0 elsewhere.

Per window h (4 full windows of 65536 px + one 2560-px tail):
  - DMA: 19 contiguous 128KB loads (class c rows)
  - ACT: E = exp(L) bf16, batched 4 classes per instruction
  - PE : S = sum_c E via identity-matmul PSUM accumulation
  - DVE: r = approx-recip(S) f32 -> bf16 (partition-aligned, no broadcast)
  - DVE: W_c = E_c * r (tensor_tensor bf16, 2x mode)
  - PE : colsum matmuls (onescol_c lhsT) into per-window [19, 512] PSUM
  - DMA out: [19, 512] f32 per window -> out [19, 2068]
Host: per-class column-range sums + dummy corrections + dice.
"""

import sys

import numpy as np

sys.path.insert(0, "/opt/trn_rl_repo")

import ml_dtypes  # noqa: E402

B, C, H, W = 8, 19, 512, 512
HW = H * W  # 262144
IGNORE_INDEX = 255
SMOOTH = 1.0

F = 512  # max free-dim columns per tile (PSUM bank)
# uneven window widths (in 128-px column units): big windows first for
# DMA/compute overlap, small windows last for a short pipeline drain
WFS = [512, 512, 512, 384, 128, 20]
UCOLS = sum(WFS)  # 2068 total 128-px column units
NP = UCOLS * 128  # 264704 padded pixels
CONST_COLS = 128 + C * C

_CACHE = {}


def _host_consts():
    """identity [128,128] + per-class ones-column lhsT variants [128,19]."""
    bf16 = ml_dtypes.bfloat16
    cb = np.zeros((128, CONST_COLS), dtype=bf16)
    cb[:, 0:128] = np.eye(128, dtype=bf16)
    for c in range(C):
        cb[:, 128 + C * c + c] = 1
    return (cb,)


def _class_layout(t_flat):
    """Segment layout for one core: counts, per-class pads, column starts.

    Returns (counts, pad, ucol_start, ucol_len, ps_corr) where ucol_* are in
    128-px column units and ps_corr[c] = exact dummy mass to subtract from
    PS[c].
    """
    valid = t_flat != IGNORE_INDEX
    counts = np.bincount(t_flat[valid].astype(np.int64), minlength=C)[:C]
    pad = (-counts) % 128
    seg = counts + pad
    starts_px = np.concatenate([[0], np.cumsum(seg)])
    assert starts_px[-1] <= NP
    ucol_start = starts_px[:-1] // 128
    ucol_len = seg // 128
    ps_corr = np.zeros(C, dtype=np.float64)
    for c in range(C):
        ps_corr[(c + 1) % C] += pad[c]  # segment-c dummies dump on class c+1
    ps_corr[0] += NP - starts_px[-1]  # trailing dummies dump on class 0
    return counts, pad, ucol_start, ucol_len, ps_corr


def _prep_core(logits_b, t_flat):
    """Sorted+padded device arrays for one core: (main [9728,512], tail [128,380])."""
    bf16 = ml_dtypes.bfloat16
    counts, pad, _, _, _ = _class_layout(t_flat)
    order = np.argsort(t_flat, kind="stable")  # class-sorted; ignored last
    Ls = logits_b.reshape(C, HW)[:, order]

    Lp = np.full((C, NP), -10.0, dtype=np.float32)
    Lp[0, :] = 10.0  # trail default: all dummy mass on class 0
    pos_out = 0
    pos_in = 0
    for c in range(C):
        n = int(counts[c])
        Lp[:, pos_out : pos_out + n] = Ls[:, pos_in : pos_in + n]
        p = int(pad[c])
        if p:
            reg = slice(pos_out + n, pos_out + n + p)
            Lp[:, reg] = -10.0
            Lp[(c + 1) % C, reg] = 10.0
        pos_out += n + p
        pos_in += n

    # per window: rows p, cols (c, f) -> class-group DMA chunks are
    # contiguous column ranges of a [128, C*Fh] block
    blocks = []
    u0 = 0
    for fh in WFS:
        px0, px1 = u0 * 128, (u0 + fh) * 128
        blocks.append(
            Lp[:, px0:px1]
            .reshape(C, fh, 128)
            .transpose(2, 0, 1)
            .reshape(128, C * fh)
            .astype(bf16)
        )
        u0 += fh
    return blocks


def _build_program():
    import concourse.bacc as bacc
    import concourse.mybir as mybir
    import concourse.tile as tile

    dt = mybir.dt
    AOP = mybir.AluOpType
    ACTF = mybir.ActivationFunctionType

    nc = bacc.Bacc("TRN2", target_bir_lowering=False, debug=False)
    win_d = [
        nc.declare_dram_parameter(f"logits_w{h}", [128, C * fh], dt.bfloat16, isOutput=False)
        for h, fh in enumerate(WFS)
    ]
    cb_d = nc.declare_dram_parameter(
        "consts_bf", [128, CONST_COLS], dt.bfloat16, isOutput=False
    )
    out_d = nc.declare_dram_parameter("out", [C, UCOLS], dt.float32, isOutput=True)

    NH = len(WFS)
    U0 = [sum(WFS[:h]) for h in range(NH)]
    GRP = 4  # classes per exp / W-multiply group

    with tile.TileContext(nc) as tc:
        with (
            tc.tile_pool(name="singles", bufs=1) as sing,
            tc.tile_pool(name="Ew", bufs=4) as Ewp,
            tc.tile_pool(name="Rp", bufs=2) as Rp,
            tc.tile_pool(name="Wp", bufs=2) as Wp,
            tc.tile_pool(name="psS", bufs=3, space="PSUM") as psS,
            tc.tile_pool(name="psW", bufs=2, space="PSUM") as psWp,
        ):
            # preload the Exp activation table off the critical path
            dumm = sing.tile([1, 2], dt.bfloat16)
            nc.vector.memset(dumm[:], 0.0)
            nc.scalar.activation(dumm[:], dumm[:], ACTF.Exp)

            consts = sing.tile([128, CONST_COLS], dt.bfloat16)
            nc.sync.dma_start(consts[:], cb_d[:])
            ident = consts[0:128, 0:128]
            onescol = [consts[0:128, 128 + C * c : 128 + C * (c + 1)] for c in range(C)]
            owb = sing.tile([C, UCOLS], dt.float32)

            Es = [None] * NH
            SPs = [None] * NH
            Rbs = [None] * NH

            def setup(h):
                """Chunked DMA + in-place exp + S accumulation for window h."""
                f = WFS[h]
                E = Ewp.tile([128, C * F], dt.bfloat16, tag="E")
                SP = psS.tile([128, F], dt.float32, tag="S")
                for c0 in range(0, C, GRP):
                    c1 = min(c0 + GRP, C)
                    nc.sync.dma_start(
                        E[:, c0 * f : c1 * f], win_d[h][:, c0 * f : c1 * f]
                    )
                    nc.scalar.activation(
                        E[:, c0 * f : c1 * f], E[:, c0 * f : c1 * f], ACTF.Exp
                    )
                for c in range(C):
                    nc.tensor.matmul(
                        SP[:, 0:f],
                        ident,
                        E[:, c * f : (c + 1) * f],
                        start=(c == 0),
                        stop=(c == C - 1),
                    )
                Es[h], SPs[h] = E, SP

            def recip(h):
                f = WFS[h]
                Rf = Rp.tile([128, F], dt.float32, tag="Rf")
                nc.vector.reciprocal_approx_fast(Rf[:, 0:f], SPs[h][:, 0:f])
                Rb = Rp.tile([128, F], dt.bfloat16, tag="Rb")
                nc.vector.tensor_copy(Rb[:, 0:f], Rf[:, 0:f])
                Rbs[h] = Rb

            def colsum(h):
                f = WFS[h]
                E, Rb = Es[h], Rbs[h]
                psW = psWp.tile([C, F], dt.float32, tag="psW")
                for c0 in range(0, C, GRP):
                    c1 = min(c0 + GRP, C)
                    g = c1 - c0
                    Wt = Wp.tile([128, GRP * F], dt.bfloat16, tag="W")
                    nc.vector.tensor_tensor(
                        out=Wt[:, 0 : g * f].rearrange("p (g f) -> p g f", f=f),
                        in0=E[:, c0 * f : c1 * f].rearrange("p (g f) -> p g f", f=f),
                        in1=Rb[:, 0:f].unsqueeze(1).broadcast_to([128, g, f]),
                        op=AOP.mult,
                    )
                    for c in range(c0, c1):
                        nc.tensor.matmul(
                            psW[:, 0:f],
                            onescol[c],
                            Wt[:, (c - c0) * f : (c - c0 + 1) * f],
                            start=(c == 0),
                            stop=(c == C - 1),
                        )
                u0 = U0[h]
                nc.vector.tensor_copy(owb[:, u0 : u0 + f], psW[:, 0:f])
                nc.sync.dma_start(out_d[0:C, u0 : u0 + f], owb[:, u0 : u0 + f])

            # software pipeline: stay 2-3 setups ahead of the colsum pass so
            # PE never waits on the DVE recip/W chain; small windows last for
            # a short drain
            setup(0)
            setup(1)
            setup(2)
            recip(0)
            for h in range(NH):
                colsum(h)
                if h + 3 < NH:
                    setup(h + 3)
                if h + 1 < NH:
                    recip(h + 1)

    nc.compile()
    return nc


def _get_program():
    if "nc" not in _CACHE:
        _CACHE["nc"] = _build_program()
        _CACHE["consts"] = _host_consts()
    return _CACHE["nc"], _CACHE["consts"]


def _install_ntff_hook():
    """antenv.axon_hooks is missing in this image; synthesize it so
    run_bass_kernel_spmd(trace=True) can capture NTFF profiles via axon."""
    import types

    if "antenv.axon_hooks" in sys.modules:
        return
    mod = types.ModuleType("antenv.axon_hooks")
    _h = [None]
    mod.set_axon_ntff_profile_hook = lambda h: _h.__setitem__(0, h)
    mod.get_axon_ntff_profile_hook = lambda: _h[0]
    sys.modules["antenv.axon_hooks"] = mod
    import antenv

    antenv.axon_hooks = mod
    from trn_agent_boot.trn_boot import _ntff_profile_via_ctypes

    mod.set_axon_ntff_profile_hook(
        _ntff_profile_via_ctypes("/opt/axon/libaxon_pjrt.so")
    )


def _run_device(logits_np, targets_np, trace=False):
    """Run the SPMD kernel on 8 cores; returns (list of out arrays, results obj)."""
    from concourse.bass_utils import run_bass_kernel_spmd

    nc, (cb,) = _get_program()
    logits_np = np.asarray(logits_np, dtype=np.float32)
    targets_np = np.asarray(targets_np)
    in_maps = []
    for b in range(B):
        blocks = _prep_core(logits_np[b], targets_np[b].reshape(-1))
        m = {f"logits_w{h}": blk for h, blk in enumerate(blocks)}
        m["consts_bf"] = cb
        in_maps.append(m)
    kwargs = {}
    if trace:
        _install_ntff_hook()
        kwargs = {"trace": True, "trace_cores": [0]}
    res = run_bass_kernel_spmd(nc, in_maps, core_ids=list(range(B)), **kwargs)
    outs = [res.results[b]["out"] for b in range(B)]
    return outs, res


def _combine(outs, targets_np):
    targets_np = np.asarray(targets_np)
    t_all = targets_np.reshape(-1)
    valid_all = t_all != IGNORE_INDEX
    if not valid_all.any():
        return np.asarray(0.0, dtype=np.float32)
    PS = np.zeros(C, dtype=np.float64)
    I = np.zeros(C, dtype=np.float64)
    for b, o in enumerate(outs):
        psw = o.astype(np.float64)  # [C, UCOLS] per-column sums of W_c
        t_flat = targets_np[b].reshape(-1)
        _, _, ustart, ulen, ps_corr = _class_layout(t_flat)
        PS += psw.sum(axis=1) - ps_corr
        for c in range(C):
            I[c] += psw[c, ustart[c] : ustart[c] + ulen[c]].sum()
    CT = np.bincount(t_all[valid_all].astype(np.int64), minlength=C)[:C].astype(
        np.float64
    )
    dice = (2.0 * I + SMOOTH) / (PS + CT + SMOOTH)
    loss = (1.0 - dice).mean()
    return np.asarray(loss, dtype=np.float32)


def kernel(logits, targets):
    logits = np.asarray(logits)
    targets = np.asarray(targets)
    outs, _ = _run_device(logits, targets)
    return _combine(outs, targets)
